# revision 1
# baseline (speedup 1.0000x reference)
"""Trainium2 Bass kernel for EquivariantPPFAttention (gnn_message_passing).

Contract: kernel(**inputs) takes FULL unsharded inputs (as produced by
reference.setup_inputs()) and returns the FULL [N, OUT, 3] float32 output.

Strategy (data-parallel over query points N across 8 NeuronCores):
  - shard q_pts / neighbor_indices across cores; replicate everything else.
  - one combined gather table comb[M, 512B]: s_feats row in bf16 (384B) +
    s_pts/normals in f32 (24B) + pad. Per query-tile of 128, dma_gather
    pulls all 128*32 neighbor rows (4 gathers of 1024 idxs - HW limit).
  - K-sum of the bf16 s_feats part on DVE (strided reduce, f32 accum);
    pts/normals extracted to a packed buffer for the PPF phase.
  - PPF angles via DVE arithmetic + ACT Sqrt/Arctan
    (atan2(r,y) = atan(r/y) + pi*[y<0] for r>=0).
  - tiny MLP on TensorE with rows on the free dim; two query-tiles packed
    per matmul via block-diagonal weights; mean-over-K folded into W3; the
    1/pi PPF normalization folded into W1; 1/K of the value path folded
    into Wv.
"""

import math
import numpy as np

N = 20000
M = 20000
K = 32
D = 64
HID = 64
OUT = 192
PPF_OUT = 64
N_CORES = 8
PI = math.pi

ES = 128          # f32 elems per comb row (512 B)
SFW = 96          # f32 slots holding the 192 bf16 s_feats values
PNO = 96          # f32 slot offset of pts/normals (6 floats)
NI = 1024         # idxs per dma_gather (HW-stable limit)
GPT = (128 * K) // NI   # gathers per query tile (4)
KPG = K // GPT    # k-blocks per gather (8)

_NC_CACHE = {}


def _build_nc(T, stage="full", loop=None):
    """Per-core Bass program for T query-tiles of 128.

    stage: debug bisection point - gather | ppf | mlp | full.
    loop: if set, repeat the whole body N times (for timing).
    """
    from contextlib import ExitStack, nullcontext
    from concourse import bacc, bass, mybir, tile

    assert T % 2 == 0
    NPAIR = T // 2
    NQ = 128 * T
    f32 = mybir.dt.float32
    bf16 = mybir.dt.bfloat16
    i16 = mybir.dt.int16
    AF = mybir.ActivationFunctionType
    ALU = mybir.AluOpType

    nc = bacc.Bacc("TRN2", target_bir_lowering=False, debug=False)

    comb_in = nc.dram_tensor("comb", [M, ES], f32, kind="ExternalInput")
    qp_in = nc.dram_tensor("qp", [128, T, 3], f32, kind="ExternalInput")
    idx_in = nc.dram_tensor("idx16", [128, T, GPT, NI // 16], i16,
                            kind="ExternalInput")
    w1b_in = nc.dram_tensor("w1b", [8, 128], f32, kind="ExternalInput")
    b1b_in = nc.dram_tensor("b1b", [128, 1], f32, kind="ExternalInput")
    w2b_in = nc.dram_tensor("w2b", [128, 128], f32, kind="ExternalInput")
    b2b_in = nc.dram_tensor("b2b", [128, 1], f32, kind="ExternalInput")
    w3b_in = nc.dram_tensor("w3b", [128, 128], f32, kind="ExternalInput")
    b3b_in = nc.dram_tensor("b3b", [128, 1], f32, kind="ExternalInput")
    wgb_in = nc.dram_tensor("wgb", [128, 3, 128], f32, kind="ExternalInput")
    bgb_in = nc.dram_tensor("bgb", [128, 3], f32, kind="ExternalInput")
    wvb_in = nc.dram_tensor("wvb", [128, 3, 128], f32, kind="ExternalInput")
    ident_in = nc.dram_tensor("ident", [128, 128], f32, kind="ExternalInput")

    if stage == "full":
        out_dev = nc.dram_tensor("out", [3, OUT, NQ], f32, kind="ExternalOutput")
        dbg = None
    else:
        DBGW = {
            "gather": T * K * 8 + T * 192,
            "ppf": 4 * T * K,
            "mlp": 128 * K + 128 + 128 + 3 * 128,
        }[stage]
        out_dev = None
        dbg = nc.dram_tensor("dbg", [128, DBGW], f32, kind="ExternalOutput")

    with tile.TileContext(nc) as tc, ExitStack() as ctx:
        const = ctx.enter_context(tc.tile_pool(name="const", bufs=1))
        gpool = ctx.enter_context(tc.tile_pool(name="gpool", bufs=2))
        gath = ctx.enter_context(tc.tile_pool(name="gath", bufs=1))
        planes = ctx.enter_context(tc.tile_pool(name="planes", bufs=1))
        temps = ctx.enter_context(tc.tile_pool(name="temps", bufs=1))
        mlpp = ctx.enter_context(tc.tile_pool(name="mlpp", bufs=1))
        small = ctx.enter_context(tc.tile_pool(name="small", bufs=2))
        psmlp = ctx.enter_context(tc.tile_pool(name="psmlp", bufs=3, space="PSUM"))
        pssm = ctx.enter_context(tc.tile_pool(name="pssm", bufs=4, space="PSUM"))

        def cload(name, dram, shape, dt=f32):
            t = const.tile(shape, dt, tag=name, name=name)
            if len(shape) > 3:
                dims = " ".join(f"d{i}" for i in range(len(shape) - 1))
                pat = f"p {dims} -> p ({dims})"
                nc.sync.dma_start(t[:].rearrange(pat), dram.ap().rearrange(pat))
            else:
                nc.sync.dma_start(t[:], dram.ap())
            return t

        qp_t = cload("qp", qp_in, [128, T, 3])
        idx_t = cload("idx16", idx_in, [128, T, GPT, NI // 16], i16)
        w1b_t = cload("w1b", w1b_in, [8, 128])
        b1b_t = cload("b1b", b1b_in, [128, 1])
        w2b_t = cload("w2b", w2b_in, [128, 128])
        b2b_t = cload("b2b", b2b_in, [128, 1])
        w3b_t = cload("w3b", w3b_in, [128, 128])
        b3b_t = cload("b3b", b3b_in, [128, 1])
        wgb_t = cload("wgb", wgb_in, [128, 3, 128])
        bgb_t = cload("bgb", bgb_in, [128, 3])
        wvb_t = cload("wvb", wvb_in, [128, 3, 128])
        ident_t = cload("ident", ident_in, [128, 128])

        _loop_ctx = tc.For_i(0, loop, 1) if loop else nullcontext()
        with _loop_ctx:
            # ---- gather + per-tile K-reduce + pn extraction ----
            nbbuf = gath.tile([128, T, K, 8], f32, tag="nbbuf")
            sfsum = gath.tile([128, T, 192], f32, tag="sfsum")

            for t in range(T):
                gt = gpool.tile([128, K, ES], f32, tag="gt", name="gt")
                for g in range(GPT):
                    nc.gpsimd.dma_gather(
                        out_ap=gt[:, g * KPG : (g + 1) * KPG, :],
                        in_ap=comb_in.ap(),
                        idxs_ap=idx_t[:, t, g, :],
                        num_idxs=NI,
                        num_idxs_reg=NI,
                        elem_size=ES,
                    )
                # K-sum of the bf16 s_feats block: view (p, e, k), reduce k
                gtb = gt[:].bitcast(bf16)          # [128, K, 256]
                red_in = gtb.rearrange("p k e -> p e k")[:, 0:192, :]
                nc.vector.reduce_sum(sfsum[:, t, :], red_in, mybir.AxisListType.X)
                # pts/normals (f32) -> nbbuf[:, t, :, 0:6]
                nc.vector.tensor_copy(
                    nbbuf[:, t, :, 0:6], gt[:, :, PNO : PNO + 6]
                )

            if stage == "gather":
                nc.sync.dma_start(
                    dbg.ap()[:, : T * K * 8],
                    nbbuf[:].rearrange("p t k c -> p (t k c)"),
                )
                nc.sync.dma_start(
                    dbg.ap()[:, T * K * 8 :],
                    sfsum[:].rearrange("p t c -> p (t c)"),
                )
            else:
                # ---- PPF geometric features (core-wide [128, T*K] planes) ----
                FW = T * K

                def ptile(tag):
                    return planes.tile([128, FW], f32, tag=tag, name=tag)

                def ttile(tag):
                    return temps.tile([128, FW], f32, tag=tag, name=tag)

                def np_c(c):
                    return nbbuf[:, :, :, c]

                def nn_c(c):
                    return nbbuf[:, :, :, 3 + c]

                def qn_c(c):
                    return nbbuf[:, :, 0, 3 + c].to_broadcast([128, T, K])

                def qp_c(c):
                    return qp_t[:, :, c].to_broadcast([128, T, K])

                def v3(t_):
                    return t_[:].rearrange("p (t k) -> p t k", k=K)

                TT = nc.vector.tensor_tensor
                STT = nc.vector.scalar_tensor_tensor

                vd = []
                for c in range(3):
                    t_ = ttile(f"vd{c}")
                    TT(v3(t_), np_c(c), qp_c(c), ALU.subtract)
                    vd.append(t_)

                def dot_views(av, bv, out_tag):
                    m0 = ttile("dm0")
                    TT(m0[:], av[0], bv[0], ALU.mult)
                    m1 = ttile("dm1")
                    TT(m1[:], av[1], bv[1], ALU.mult)
                    s = ttile(out_tag)
                    TT(s[:], m0[:], m1[:], ALU.add)
                    m2 = ttile("dm0")
                    TT(m2[:], av[2], bv[2], ALU.mult)
                    TT(s[:], s[:], m2[:], ALU.add)
                    return s

                def cross_views(av, bv):
                    outs = []
                    for c in range(3):
                        i, j = (c + 1) % 3, (c + 2) % 3
                        m0 = ttile("cm0")
                        TT(m0[:], av[i], bv[j], ALU.mult)
                        m1 = ttile("cm1")
                        TT(m1[:], av[j], bv[i], ALU.mult)
                        o = ttile(f"cr{c}")
                        TT(o[:], m0[:], m1[:], ALU.subtract)
                        outs.append(o)
                    return outs

                vdv = [v3(t_) for t_ in vd]
                qnv = [qn_c(c) for c in range(3)]
                nnv = [nn_c(c) for c in range(3)]

                dd = dot_views(vdv, vdv, "y_")
                d_pl = ptile("d_pl")
                nc.scalar.activation(d_pl[:], dd[:], AF.Sqrt)

                def angle_plane(av, bv, tag):
                    y = dot_views(av, bv, "y_")
                    cr = cross_views(av, bv)
                    crv = [c_[:] for c_ in cr]
                    rs = dot_views(crv, crv, "rs_")
                    r = ttile("cm0")
                    nc.scalar.activation(r[:], rs[:], AF.Sqrt)
                    iy = ttile("cm1")
                    nc.vector.reciprocal(iy[:], y[:])
                    tq = ttile("dm0")
                    TT(tq[:], r[:], iy[:], ALU.mult)
                    at = ttile("dm1")
                    nc.scalar.activation(at[:], tq[:], AF.Arctan)
                    ind = ttile("cr1")
                    nc.vector.tensor_scalar(ind[:], y[:], 0.0, None, ALU.is_lt)
                    pl = ptile(tag)
                    STT(pl[:], ind[:], PI, at[:], ALU.mult, ALU.add)
                    return pl

                a1_pl = angle_plane(qnv, vdv, "a1_pl")
                a2_pl = angle_plane(nnv, vdv, "a2_pl")
                a3_pl = angle_plane(qnv, nnv, "a3_pl")
                plane_list = [d_pl, a1_pl, a2_pl, a3_pl]

                if stage == "ppf":
                    for ci in range(4):
                        nc.sync.dma_start(
                            dbg.ap()[:, ci * FW : (ci + 1) * FW], plane_list[ci][:]
                        )
                else:
                    if stage == "full":
                        out_re = out_dev.ap().rearrange(
                            "c (jj p) q -> p c jj q", jj=3
                        )

                    RW = 128 * K  # rows per query-tile (4096)
                    HC = RW // 2
                    npair_run = 1 if stage == "mlp" else NPAIR
                    for j in range(npair_run):
                        pf = mlpp.tile([8, RW], f32, tag="pf", bufs=2)
                        for t2 in range(2):
                            t_abs = 2 * j + t2
                            for ci in range(4):
                                nc.sync.dma_start(
                                    pf[t2 * 4 + ci : t2 * 4 + ci + 1, :],
                                    plane_list[ci][:, t_abs * K : (t_abs + 1) * K],
                                )

                        ksum = small.tile([128, 128], f32, tag="ksum")
                        for hh in range(2):
                            h1s = mlpp.tile([128, HC], f32, tag="h1s", bufs=2)
                            for ch in range(HC // 512):
                                sl = slice(ch * 512, (ch + 1) * 512)
                                slg = slice(hh * HC + ch * 512, hh * HC + (ch + 1) * 512)
                                h1p = psmlp.tile([128, 512], f32, tag="psmlp")
                                nc.tensor.matmul(
                                    h1p[:], w1b_t[:], pf[:, slg], start=True, stop=True
                                )
                                nc.scalar.activation(
                                    h1s[:, sl], h1p[:], AF.Relu, bias=b1b_t[:]
                                )
                            h2s = mlpp.tile([128, HC], f32, tag="h2s", bufs=2)
                            for ch in range(HC // 512):
                                sl = slice(ch * 512, (ch + 1) * 512)
                                h2p = psmlp.tile([128, 512], f32, tag="psmlp")
                                nc.tensor.matmul(
                                    h2p[:], w2b_t[:], h1s[:, sl], start=True, stop=True
                                )
                                nc.vector.tensor_scalar(
                                    h2s[:, sl], h2p[:], b2b_t[:], 0.0, ALU.add, ALU.max
                                )
                            nc.vector.reduce_sum(
                                ksum[:, hh * 64 : (hh + 1) * 64],
                                h2s[:].rearrange("p (q k) -> p q k", k=K),
                                mybir.AxisListType.X,
                            )

                        pmp = pssm.tile([128, 128], f32, tag="pssm")
                        nc.tensor.matmul(pmp[:], w3b_t[:], ksum[:], start=True, stop=True)
                        pms = small.tile([128, 128], f32, tag="pms")
                        nc.vector.tensor_scalar_add(pms[:], pmp[:], b3b_t[:])

                        if stage == "mlp" and j == 0:
                            nc.sync.dma_start(dbg.ap()[:, : RW // 2], h2s[:])
                            nc.sync.dma_start(dbg.ap()[:, RW : RW + 128], ksum[:])
                            nc.sync.dma_start(dbg.ap()[:, RW + 128 : RW + 256], pms[:])

                        gates = []
                        for jj in range(3):
                            gp = pssm.tile([128, 128], f32, tag="pssm")
                            nc.tensor.matmul(
                                gp[:], wgb_t[:, jj, :], pms[:], start=True, stop=True
                            )
                            gs = small.tile(
                                [128, 128], f32, tag=f"gate{jj}", name=f"gate{jj}"
                            )
                            nc.scalar.activation(
                                gs[:], gp[:], AF.Sigmoid, bias=bgb_t[:, jj : jj + 1]
                            )
                            gates.append(gs)
                            if stage == "mlp" and j == 0:
                                nc.sync.dma_start(
                                    dbg.ap()[
                                        :,
                                        RW + 256 + jj * 128 : RW + 256 + (jj + 1) * 128,
                                    ],
                                    gs[:],
                                )
                        if stage == "mlp":
                            continue

                        # value path: one transpose per component covers both
                        # tiles of the pair:
                        # in [128 q, (2 t x 64 d)] -> out [(2 t x 64 d), 128 q]
                        av = sfsum[:, 2 * j : 2 * j + 2, :].rearrange(
                            "p t (d c) -> p c (t d)", c=3
                        )
                        vstage = small.tile([128, 3, 3, 128], f32, tag="vstage")
                        for c in range(3):
                            tp = pssm.tile([128, 128], f32, tag="pssm")
                            nc.tensor.transpose(tp[:], av[:, c, :], ident_t[:])
                            aggs = small.tile([128, 128], f32, tag="aggs")
                            nc.vector.tensor_copy(aggs[:], tp[:])
                            for jj in range(3):
                                vp = pssm.tile([128, 128], f32, tag="pssm")
                                nc.tensor.matmul(
                                    vp[:], wvb_t[:, jj, :], aggs[:],
                                    start=True, stop=True,
                                )
                                TT(vstage[:, c, jj, :], vp[:], gates[jj][:], ALU.mult)

                        for h in range(2):
                            q0 = (2 * j + h) * 128
                            nc.sync.dma_start(
                                out_re[:, :, :, q0 : q0 + 128].rearrange(
                                    "p c jj q -> p (c jj) q"
                                ),
                                vstage[h * 64 : (h + 1) * 64, :, :, :].rearrange(
                                    "p c jj q -> p (c jj) q"
                                ),
                            )

    nc.compile()
    return nc


def _f32_to_bf16_bits(x):
    """Round-to-nearest-even f32 -> bf16, returned as uint16 bits."""
    u = np.ascontiguousarray(x, dtype=np.float32).view(np.uint32)
    rounded = (u + 0x7FFF + ((u >> 16) & 1)) >> 16
    return rounded.astype(np.uint16)


def _host_prep(q_pts, s_pts, s_feats, neighbor_indices, normals,
               W1, b1, W2, b2, W3, b3, Wg, bg, Wv, T, n_total=N):
    NQ = 128 * T
    n_per_core = n_total // N_CORES
    f = np.float32

    comb = np.zeros((M, ES), dtype=f)
    cb = comb.view(np.uint16).reshape(M, ES * 2)
    cb[:, : 2 * SFW] = _f32_to_bf16_bits(s_feats.reshape(M, 192))
    comb[:, PNO : PNO + 3] = s_pts
    comb[:, PNO + 3 : PNO + 6] = normals

    W1T = W1.T.astype(f).copy()
    W1T[1:4] *= f(1.0 / PI)
    w1b = np.zeros((8, 128), dtype=f)
    w1b[0:4, 0:64] = W1T
    w1b[4:8, 64:128] = W1T
    b1b = np.concatenate([b1, b1]).astype(f)[:, None]

    def blockdiag2(A):
        n_, m_ = A.shape
        o = np.zeros((2 * n_, 2 * m_), dtype=f)
        o[:n_, :m_] = A
        o[n_:, m_:] = A
        return o

    w2b = blockdiag2(W2.T.astype(f))
    b2b = np.concatenate([b2, b2]).astype(f)[:, None]
    w3b = blockdiag2((W3.T / K).astype(f))
    b3b = np.concatenate([b3, b3]).astype(f)[:, None]

    WgT = Wg.T.astype(f)
    WvT = (Wv.T / K).astype(f)
    wgb = np.zeros((3, 128, 128), dtype=f)
    wvb = np.zeros((3, 128, 128), dtype=f)
    bgb = np.zeros((128, 3), dtype=f)
    for jj in range(3):
        wgb[jj] = blockdiag2(WgT[:, jj * 64 : (jj + 1) * 64])
        wvb[jj] = blockdiag2(WvT[:, jj * 64 : (jj + 1) * 64])
        bgb[:, jj] = np.concatenate([bg[jj * 64 : (jj + 1) * 64]] * 2)
    wgb_host = np.ascontiguousarray(wgb.transpose(1, 0, 2))
    wvb_host = np.ascontiguousarray(wvb.transpose(1, 0, 2))
    ident = np.eye(128, dtype=f)

    shared = dict(
        comb=comb, w1b=w1b, b1b=b1b, w2b=w2b, b2b=b2b, w3b=w3b, b3b=b3b,
        wgb=wgb_host, bgb=bgb, wvb=wvb_host, ident=ident,
    )

    in_maps = []
    for i in range(N_CORES):
        lo = i * n_per_core
        hi = lo + n_per_core
        qp_pad = np.zeros((NQ, 3), dtype=f)
        qp_pad[: hi - lo] = q_pts[lo:hi]
        idx_pad = np.zeros((NQ, K), dtype=np.int64)
        idx_pad[: hi - lo] = neighbor_indices[lo:hi]

        qp_host = np.ascontiguousarray(qp_pad.reshape(T, 128, 3).transpose(1, 0, 2))

        # idx16[p, t, g, s]: gather g of tile t covers logical rows
        # i' = (k - g*KPG)*128 + q, wrapped: w[l, s] = list[s*16 + l]
        idx16 = np.zeros((128, T, GPT, NI // 16), np.int16)
        for t in range(T):
            arr = idx_pad[t * 128 : (t + 1) * 128, :]      # [128 q, K]
            for g in range(GPT):
                lst = arr[:, g * KPG : (g + 1) * KPG].T.reshape(NI)
                idx16[:, t, g, :] = np.tile(
                    lst.reshape(NI // 16, 16).T.astype(np.int16), (8, 1)
                )

        m = dict(shared)
        m.update(qp=qp_host, idx16=idx16)
        in_maps.append(m)
    return in_maps


def kernel(**inputs):
    from concourse.bass_utils import run_bass_kernel_spmd

    T = 20
    inputs = {k: np.asarray(v) for k, v in inputs.items()}
    idx = inputs["neighbor_indices"].astype(np.int64)

    if T not in _NC_CACHE:
        _NC_CACHE[T] = _build_nc(T)
    nc = _NC_CACHE[T]

    in_maps = _host_prep(
        inputs["q_pts"], inputs["s_pts"], inputs["s_feats"], idx,
        inputs["normals"], inputs["W1"], inputs["b1"], inputs["W2"],
        inputs["b2"], inputs["W3"], inputs["b3"], inputs["Wg"],
        inputs["bg"], inputs["Wv"], T,
    )
    res = run_bass_kernel_spmd(nc, in_maps, core_ids=list(range(N_CORES)))

    n_per_core = N // N_CORES
    out = np.empty((N, OUT, 3), dtype=np.float32)
    for i in range(N_CORES):
        o = res.results[i]["out"]
        out[i * n_per_core : (i + 1) * n_per_core] = o.transpose(2, 1, 0)[:n_per_core]
    return out



# revision 3
# speedup vs baseline: 2.0271x; 2.0271x over previous
"""Trainium2 Bass kernel for EquivariantPPFAttention (gnn_message_passing).

Contract: kernel(**inputs) takes FULL unsharded inputs (as produced by
reference.setup_inputs()) and returns the FULL [N, OUT, 3] float32 output.

Strategy (data-parallel over query points N across 8 NeuronCores):
  - shard q_pts / neighbor_indices across cores; replicate everything else.
  - one combined gather table comb[M, 512B]: s_feats row in bf16 (384B) +
    s_pts/normals in f32 (24B) + pad. dma_gather pulls 128*32 neighbor rows
    per query tile as 4 gathers of 1024 idxs, spread round-robin over 4
    SWDGE queues (descriptor generation runs on different Q7 core pairs
    concurrently -> ~2.7x faster than one queue).
  - fully pipelined per PAIR of query tiles: gather pair j+1 while pair j
    runs K-sum (bf16 tree adds on DVE), PPF geometry (DVE + ACT), the tiny
    MLP (TensorE, bf16), and the gated value path.
  - PPF angles: atan2(r,y) = atan(r/y) + pi/2 - pi/2*sign(y); the constant
    pi/2 term is folded into b1, the 1/pi normalization into W1, mean-over-K
    into W3, and 1/K of the value path into Wv.
  - two query-tiles packed per matmul via block-diagonal weights.
"""

import math
import numpy as np
import ml_dtypes

N = 20000
M = 20000
K = 32
D = 64
HID = 64
OUT = 192
PPF_OUT = 64
N_CORES = 8
PI = math.pi

ES = 128          # f32 elems per comb row (512 B)
SFW = 96          # f32 slots holding the 192 bf16 s_feats values
PNO = 96          # f32 slot offset of pts/normals (6 floats)
NI = 1024         # idxs per dma_gather (HW-stable limit)
GPT = (128 * K) // NI   # gathers per query tile (4)
KPG = K // GPT    # k-blocks per gather (8)
NQ_SW = 4         # SWDGE queues used round-robin

_NC_CACHE = {}


def _build_nc(T):
    """Per-core Bass program for T query-tiles of 128 (T even)."""
    from contextlib import ExitStack
    from concourse import bacc, bass, mybir, tile

    assert T % 2 == 0
    NPAIR = T // 2
    NQ = 128 * T
    f32 = mybir.dt.float32
    bf16 = mybir.dt.bfloat16
    i16 = mybir.dt.int16
    AF = mybir.ActivationFunctionType
    ALU = mybir.AluOpType

    nc = bacc.Bacc("TRN2", target_bir_lowering=False, debug=False,
                   num_swdge_queues=NQ_SW)

    comb_in = nc.dram_tensor("comb", [M, ES], f32, kind="ExternalInput")
    qp_in = nc.dram_tensor("qp", [128, T, 3], f32, kind="ExternalInput")
    idx_in = nc.dram_tensor("idx16", [128, T, GPT, NI // 16], i16,
                            kind="ExternalInput")
    w1b_in = nc.dram_tensor("w1b", [8, 128], bf16, kind="ExternalInput")
    b1b_in = nc.dram_tensor("b1b", [128, 1], f32, kind="ExternalInput")
    w2b_in = nc.dram_tensor("w2b", [128, 128], bf16, kind="ExternalInput")
    b2b_in = nc.dram_tensor("b2b", [128, 1], f32, kind="ExternalInput")
    w3b_in = nc.dram_tensor("w3b", [128, 128], f32, kind="ExternalInput")
    b3b_in = nc.dram_tensor("b3b", [128, 1], f32, kind="ExternalInput")
    wgb_in = nc.dram_tensor("wgb", [128, 3, 128], f32, kind="ExternalInput")
    bgb_in = nc.dram_tensor("bgb", [128, 3], f32, kind="ExternalInput")
    wvb_in = nc.dram_tensor("wvb", [128, 3, 128], bf16, kind="ExternalInput")
    ident_in = nc.dram_tensor("ident", [128, 128], f32, kind="ExternalInput")
    out_dev = nc.dram_tensor("out", [3, OUT, NQ], f32, kind="ExternalOutput")

    with tile.TileContext(nc) as tc, ExitStack() as ctx:
        const = ctx.enter_context(tc.tile_pool(name="const", bufs=1))
        gpool = ctx.enter_context(tc.tile_pool(name="gpool", bufs=2))
        tpool = ctx.enter_context(tc.tile_pool(name="tpool", bufs=1))
        sfpool = ctx.enter_context(tc.tile_pool(name="sfpool", bufs=2))
        pnpool = ctx.enter_context(tc.tile_pool(name="pnpool", bufs=2))
        planes = ctx.enter_context(tc.tile_pool(name="planes", bufs=2))
        temps = ctx.enter_context(tc.tile_pool(name="temps", bufs=2))
        mlpp = ctx.enter_context(tc.tile_pool(name="mlpp", bufs=1))
        small = ctx.enter_context(tc.tile_pool(name="small", bufs=2))
        psmlp = ctx.enter_context(tc.tile_pool(name="psmlp", bufs=3, space="PSUM"))
        pssm = ctx.enter_context(tc.tile_pool(name="pssm", bufs=2, space="PSUM"))

        def cload(name, dram, shape, dt=f32):
            t = const.tile(shape, dt, tag=name, name=name)
            if len(shape) > 3:
                dims = " ".join(f"d{i}" for i in range(len(shape) - 1))
                pat = f"p {dims} -> p ({dims})"
                nc.sync.dma_start(t[:].rearrange(pat), dram.ap().rearrange(pat))
            else:
                nc.sync.dma_start(t[:], dram.ap())
            return t

        qp_t = cload("qp", qp_in, [128, T, 3])
        idx_t = cload("idx16", idx_in, [128, T, GPT, NI // 16], i16)
        w1b_t = cload("w1b", w1b_in, [8, 128], bf16)
        b1b_t = cload("b1b", b1b_in, [128, 1])
        w2b_t = cload("w2b", w2b_in, [128, 128], bf16)
        b2b_t = cload("b2b", b2b_in, [128, 1])
        w3b_t = cload("w3b", w3b_in, [128, 128])
        b3b_t = cload("b3b", b3b_in, [128, 1])
        wgb_t = cload("wgb", wgb_in, [128, 3, 128])
        bgb_t = cload("bgb", bgb_in, [128, 3])
        wvb_t = cload("wvb", wvb_in, [128, 3, 128], bf16)
        ident_t = cload("ident", ident_in, [128, 128])

        out_re = out_dev.ap().rearrange("c (jj p) q -> p c jj q", jj=3)
        TT = nc.vector.tensor_tensor
        STT = nc.vector.scalar_tensor_tensor

        RW = 128 * K        # MLP rows per query tile (4096)
        HC = RW // 2        # rows per hh half (2048)
        gctr = 0

        for j in range(NPAIR):
            # ---- gather the pair's 2*128*K neighbor rows ----
            gt = gpool.tile([128, 2, K, ES], f32, tag="gt", name="gt")
            for t2 in range(2):
                for g in range(GPT):
                    nc.gpsimd.dma_gather(
                        out_ap=gt[:, t2, g * KPG : (g + 1) * KPG, :],
                        in_ap=comb_in.ap(),
                        idxs_ap=idx_t[:, 2 * j + t2, g, :],
                        num_idxs=NI,
                        num_idxs_reg=NI,
                        elem_size=ES,
                        queue_num=gctr % NQ_SW,
                    )
                    gctr += 1

            # ---- K-sum of bf16 s_feats: tree adds (contiguous reads) ----
            gtb = gt[:].bitcast(bf16)          # [128, 2, K, 256]
            s16 = tpool.tile([128, 2, 16, 192], bf16, tag="s16")
            TT(s16[:], gtb[:, :, 0:16, 0:192], gtb[:, :, 16:32, 0:192], ALU.add)
            s8 = tpool.tile([128, 2, 8, 192], bf16, tag="s8")
            TT(s8[:], s16[:, :, 0:8, :], s16[:, :, 8:16, :], ALU.add)
            s4 = tpool.tile([128, 2, 4, 192], f32, tag="s4")
            TT(s4[:], s8[:, :, 0:4, :], s8[:, :, 4:8, :], ALU.add)
            s2 = tpool.tile([128, 2, 2, 192], f32, tag="s2")
            TT(s2[:], s4[:, :, 0:2, :], s4[:, :, 2:4, :], ALU.add)
            sfs = sfpool.tile([128, 2, 192], f32, tag="sfs")
            TT(sfs[:], s2[:, :, 0, :], s2[:, :, 1, :], ALU.add)

            # ---- pack pts/normals for the pair (ACT copy) ----
            pnb = pnpool.tile([128, 2, K, 8], f32, tag="pnb")
            nc.scalar.copy(pnb[:, :, :, 0:6], gt[:, :, :, PNO : PNO + 6])

            # ---- PPF geometric features ([128, 2, K] planes) ----
            def ptile(tag):
                return planes.tile([128, 2, K], bf16, tag=tag, name=tag)

            def ttile(tag):
                return temps.tile([128, 2, K], f32, tag=tag, name=tag)

            def np_c(c):
                return pnb[:, :, :, c]

            def nn_c(c):
                return pnb[:, :, :, 3 + c]

            def qn_c(c):
                return pnb[:, :, 0, 3 + c].to_broadcast([128, 2, K])

            def qp_c(c):
                return qp_t[:, 2 * j : 2 * j + 2, c].to_broadcast([128, 2, K])

            vd = []
            for c in range(3):
                t_ = ttile(f"vd{c}")
                TT(t_[:], np_c(c), qp_c(c), ALU.subtract)
                vd.append(t_)

            def dot_views(av, bv, out_tag):
                m0 = ttile("dm0")
                TT(m0[:], av[0], bv[0], ALU.mult)
                m1 = ttile("dm1")
                TT(m1[:], av[1], bv[1], ALU.mult)
                s = ttile(out_tag)
                TT(s[:], m0[:], m1[:], ALU.add)
                m2 = ttile("dm0")
                TT(m2[:], av[2], bv[2], ALU.mult)
                TT(s[:], s[:], m2[:], ALU.add)
                return s

            def cross_views(av, bv):
                outs = []
                for c in range(3):
                    i, i2 = (c + 1) % 3, (c + 2) % 3
                    m0 = ttile("cm0")
                    TT(m0[:], av[i], bv[i2], ALU.mult)
                    m1 = ttile("cm1")
                    TT(m1[:], av[i2], bv[i], ALU.mult)
                    o = ttile(f"cr{c}")
                    TT(o[:], m0[:], m1[:], ALU.subtract)
                    outs.append(o)
                return outs

            vdv = [t_[:] for t_ in vd]
            qnv = [qn_c(c) for c in range(3)]
            nnv = [nn_c(c) for c in range(3)]

            dd = dot_views(vdv, vdv, "dd")
            ys, rss = [], []
            for (av, bv) in ((qnv, vdv), (nnv, vdv), (qnv, nnv)):
                y = dot_views(av, bv, f"y{len(ys)}")
                cr = cross_views(av, bv)
                crv = [c_[:] for c_ in cr]
                rs = dot_views(crv, crv, f"rs{len(rss)}")
                ys.append(y)
                rss.append(rs)

            # sqrt-table phase: 4 sqrts + 3 signs
            d_pl = ptile("d_pl")
            nc.scalar.activation(d_pl[:], dd[:], AF.Sqrt)
            rs_r = []
            for i in range(3):
                r = ttile(f"r{i}")
                nc.scalar.activation(r[:], rss[i][:], AF.Sqrt)
                rs_r.append(r)
            sgns = []
            for i in range(3):
                sg = ttile(f"sg{i}")
                nc.scalar.sign(sg[:], ys[i][:])
                sgns.append(sg)

            # arctan-table phase
            a_pls = [d_pl]
            for i in range(3):
                iy = ttile(f"iy{i}")
                nc.vector.reciprocal(iy[:], ys[i][:])
                tq = ttile("dm0")
                TT(tq[:], rs_r[i][:], iy[:], ALU.mult)
                at = ttile("dm1")
                nc.scalar.activation(at[:], tq[:], AF.Arctan)
                pl = ptile(f"a{i}_pl")
                STT(pl[:], sgns[i][:], -PI / 2, at[:], ALU.mult, ALU.add)
                a_pls.append(pl)

            # ---- pack planes into MLP rows: pf[8, 4096] bf16 ----
            pf = mlpp.tile([8, RW], bf16, tag="pf", bufs=2)
            for t2 in range(2):
                for ci in range(4):
                    nc.sync.dma_start(
                        pf[t2 * 4 + ci : t2 * 4 + ci + 1, :],
                        a_pls[ci][:, t2, :],
                    )

            # ---- MLP (block-diagonal 2-tile packing) ----
            ksum = small.tile([128, 128], f32, tag="ksum")
            for hh in range(2):
                h1s = mlpp.tile([128, HC], bf16, tag="h1s", bufs=2)
                for ch in range(HC // 512):
                    sl = slice(ch * 512, (ch + 1) * 512)
                    slg = slice(hh * HC + ch * 512, hh * HC + (ch + 1) * 512)
                    h1p = psmlp.tile([128, 512], f32, tag="psmlp")
                    nc.tensor.matmul(
                        h1p[:], w1b_t[:], pf[:, slg], start=True, stop=True
                    )
                    nc.scalar.activation(
                        h1s[:, sl], h1p[:], AF.Relu, bias=b1b_t[:]
                    )
                h2s = mlpp.tile([128, HC], bf16, tag="h2s", bufs=2)
                for ch in range(HC // 512):
                    sl = slice(ch * 512, (ch + 1) * 512)
                    h2p = psmlp.tile([128, 512], f32, tag="psmlp")
                    nc.tensor.matmul(
                        h2p[:], w2b_t[:], h1s[:, sl], start=True, stop=True
                    )
                    nc.scalar.activation(
                        h2s[:, sl], h2p[:], AF.Relu, bias=b2b_t[:]
                    )
                nc.vector.reduce_sum(
                    ksum[:, hh * 64 : (hh + 1) * 64],
                    h2s[:].rearrange("p (q k) -> p q k", k=K),
                    mybir.AxisListType.X,
                )

            pmp = pssm.tile([128, 128], f32, tag="pssm")
            nc.tensor.matmul(pmp[:], w3b_t[:], ksum[:], start=True, stop=True)
            pms = small.tile([128, 128], f32, tag="pms")
            nc.vector.tensor_scalar_add(pms[:], pmp[:], b3b_t[:])

            gates = []
            for jj in range(3):
                gp = pssm.tile([128, 128], f32, tag="pssm")
                nc.tensor.matmul(
                    gp[:], wgb_t[:, jj, :], pms[:], start=True, stop=True
                )
                gs = small.tile([128, 128], f32, tag=f"gate{jj}", name=f"gate{jj}")
                nc.scalar.activation(
                    gs[:], gp[:], AF.Sigmoid, bias=bgb_t[:, jj : jj + 1]
                )
                gates.append(gs)

            # ---- value path: transpose sfsum, then batched Wv matmuls ----
            av = sfs[:].rearrange("p t (d c) -> p c (t d)", c=3)
            aggs = small.tile([128, 3, 128], bf16, tag="aggs")
            for c in range(3):
                tp = pssm.tile([128, 128], f32, tag="pssm")
                nc.tensor.transpose(tp[:], av[:, c, :], ident_t[:])
                nc.scalar.copy(aggs[:, c, :], tp[:])
            vstage = small.tile([128, 3, 3, 128], f32, tag="vstage")
            for jj in range(3):
                vp = pssm.tile([128, 3, 128], f32, tag="psv")
                nc.tensor.matmul(
                    vp[:].rearrange("p c q -> p (c q)"),
                    wvb_t[:, jj, :],
                    aggs[:].rearrange("p c q -> p (c q)"),
                    start=True, stop=True,
                )
                for c in range(3):
                    TT(vstage[:, c, jj, :], vp[:, c, :], gates[jj][:], ALU.mult)

            for h in range(2):
                q0 = (2 * j + h) * 128
                nc.sync.dma_start(
                    out_re[:, :, :, q0 : q0 + 128].rearrange(
                        "p c jj q -> p (c jj) q"
                    ),
                    vstage[h * 64 : (h + 1) * 64, :, :, :].rearrange(
                        "p c jj q -> p (c jj) q"
                    ),
                )

    nc.compile()
    return nc


def _f32_to_bf16_bits(x):
    """Round-to-nearest-even f32 -> bf16, returned as uint16 bits."""
    u = np.ascontiguousarray(x, dtype=np.float32).view(np.uint32)
    rounded = (u + 0x7FFF + ((u >> 16) & 1)) >> 16
    return rounded.astype(np.uint16)


def _host_prep(q_pts, s_pts, s_feats, neighbor_indices, normals,
               W1, b1, W2, b2, W3, b3, Wg, bg, Wv, T, n_total=N):
    NQ = 128 * T
    n_per_core = n_total // N_CORES
    f = np.float32
    bf = ml_dtypes.bfloat16

    comb = np.zeros((M, ES), dtype=f)
    cb = comb.view(np.uint16).reshape(M, ES * 2)
    cb[:, : 2 * SFW] = _f32_to_bf16_bits(s_feats.reshape(M, 192))
    comb[:, PNO : PNO + 3] = s_pts
    comb[:, PNO + 3 : PNO + 6] = normals

    W1T = W1.T.astype(f).copy()
    W1T[1:4] *= f(1.0 / PI)
    w1b = np.zeros((8, 128), dtype=f)
    w1b[0:4, 0:64] = W1T
    w1b[4:8, 64:128] = W1T
    # atan2 via sign: constant pi/2 * (sum of folded angle columns) -> b1
    b1_eff = b1.astype(f) + f(PI / 2) * W1T[1:4].sum(axis=0)
    b1b = np.concatenate([b1_eff, b1_eff]).astype(f)[:, None]

    def blockdiag2(A):
        n_, m_ = A.shape
        o = np.zeros((2 * n_, 2 * m_), dtype=f)
        o[:n_, :m_] = A
        o[n_:, m_:] = A
        return o

    w2b = blockdiag2(W2.T.astype(f))
    b2b = np.concatenate([b2, b2]).astype(f)[:, None]
    w3b = blockdiag2((W3.T / K).astype(f))
    b3b = np.concatenate([b3, b3]).astype(f)[:, None]

    WgT = Wg.T.astype(f)
    WvT = (Wv.T / K).astype(f)
    wgb = np.zeros((3, 128, 128), dtype=f)
    wvb = np.zeros((3, 128, 128), dtype=f)
    bgb = np.zeros((128, 3), dtype=f)
    for jj in range(3):
        wgb[jj] = blockdiag2(WgT[:, jj * 64 : (jj + 1) * 64])
        wvb[jj] = blockdiag2(WvT[:, jj * 64 : (jj + 1) * 64])
        bgb[:, jj] = np.concatenate([bg[jj * 64 : (jj + 1) * 64]] * 2)
    wgb_host = np.ascontiguousarray(wgb.transpose(1, 0, 2))
    wvb_host = np.ascontiguousarray(wvb.transpose(1, 0, 2)).astype(bf)
    ident = np.eye(128, dtype=f)

    shared = dict(
        comb=comb, w1b=w1b.astype(bf), b1b=b1b, w2b=w2b.astype(bf), b2b=b2b,
        w3b=w3b, b3b=b3b, wgb=wgb_host, bgb=bgb, wvb=wvb_host, ident=ident,
    )

    in_maps = []
    for i in range(N_CORES):
        lo = i * n_per_core
        hi = lo + n_per_core
        qp_pad = np.zeros((NQ, 3), dtype=f)
        qp_pad[: hi - lo] = q_pts[lo:hi]
        idx_pad = np.zeros((NQ, K), dtype=np.int64)
        idx_pad[: hi - lo] = neighbor_indices[lo:hi]

        qp_host = np.ascontiguousarray(qp_pad.reshape(T, 128, 3).transpose(1, 0, 2))

        # idx16[p, t, g, s]: gather g of tile t covers logical rows
        # i' = (k - g*KPG)*128 + q, wrapped: w[l, s] = list[s*16 + l]
        idx16 = np.zeros((128, T, GPT, NI // 16), np.int16)
        for t in range(T):
            arr = idx_pad[t * 128 : (t + 1) * 128, :]      # [128 q, K]
            for g in range(GPT):
                lst = arr[:, g * KPG : (g + 1) * KPG].T.reshape(NI)
                idx16[:, t, g, :] = np.tile(
                    lst.reshape(NI // 16, 16).T.astype(np.int16), (8, 1)
                )

        m = dict(shared)
        m.update(qp=qp_host, idx16=idx16)
        in_maps.append(m)
    return in_maps


def kernel(**inputs):
    from concourse.bass_utils import run_bass_kernel_spmd

    T = 20
    inputs = {k: np.asarray(v) for k, v in inputs.items()}
    idx = inputs["neighbor_indices"].astype(np.int64)

    if T not in _NC_CACHE:
        _NC_CACHE[T] = _build_nc(T)
    nc = _NC_CACHE[T]

    in_maps = _host_prep(
        inputs["q_pts"], inputs["s_pts"], inputs["s_feats"], idx,
        inputs["normals"], inputs["W1"], inputs["b1"], inputs["W2"],
        inputs["b2"], inputs["W3"], inputs["b3"], inputs["Wg"],
        inputs["bg"], inputs["Wv"], T,
    )
    res = run_bass_kernel_spmd(nc, in_maps, core_ids=list(range(N_CORES)))

    n_per_core = N // N_CORES
    out = np.empty((N, OUT, 3), dtype=np.float32)
    for i in range(N_CORES):
        o = res.results[i]["out"]
        out[i * n_per_core : (i + 1) * n_per_core] = o.transpose(2, 1, 0)[:n_per_core]
    return out


# revision 10
# speedup vs baseline: 2.3585x; 1.1635x over previous
"""Trainium2 Bass kernel for EquivariantPPFAttention (gnn_message_passing).

Contract: kernel(**inputs) takes FULL unsharded inputs (as produced by
reference.setup_inputs()) and returns the FULL [N, OUT, 3] float32 output.

Strategy (data-parallel over query points N across 8 NeuronCores):
  - shard q_pts / neighbor_indices across cores; replicate everything else.
  - one combined gather table comb[M, 512B]: s_feats row in bf16 (384B) +
    s_pts/normals in f32 (24B) + pad. dma_gather pulls 128*32 neighbor rows
    per query tile as 4 gathers of 1024 idxs, spread round-robin over 4
    SWDGE queues (descriptor generation runs on different Q7 core pairs
    concurrently -> ~2.7x faster than one queue).
  - fully pipelined per PAIR of query tiles: gather pair j+1 while pair j
    runs K-sum (bf16 tree adds on DVE), PPF geometry (DVE + ACT), the tiny
    MLP (TensorE, bf16), and the gated value path.
  - PPF angles: atan2(r,y) = atan(r/y) + pi/2 - pi/2*sign(y); the constant
    pi/2 term is folded into b1, the 1/pi normalization into W1, mean-over-K
    into W3, and 1/K of the value path into Wv.
  - two query-tiles packed per matmul via block-diagonal weights.
"""

import math
import numpy as np
import ml_dtypes

N = 20000
M = 20000
K = 32
D = 64
HID = 64
OUT = 192
PPF_OUT = 64
N_CORES = 8
PI = math.pi

ES = 128          # f32 elems per comb row (512 B)
SFW = 96          # f32 slots holding the 192 bf16 s_feats values
PNO = 96          # f32 slot offset of pts/normals/|normal|^2 (7 floats)
NI = 1024         # idxs per dma_gather (HW-stable limit)
GPT = (128 * K) // NI   # gathers per query tile (4)
KPG = K // GPT    # k-blocks per gather (8)
NQ_SW = 4         # SWDGE queues used round-robin

_NC_CACHE = {}


def _build_nc(T):
    """Per-core Bass program for T query-tiles of 128 (T even)."""
    from contextlib import ExitStack
    from concourse import bacc, bass, mybir, tile

    assert T % 2 == 0
    NPAIR = T // 2
    NQ = 128 * T
    f32 = mybir.dt.float32
    bf16 = mybir.dt.bfloat16
    i16 = mybir.dt.int16
    AF = mybir.ActivationFunctionType
    ALU = mybir.AluOpType

    nc = bacc.Bacc("TRN2", target_bir_lowering=False, debug=False,
                   num_swdge_queues=NQ_SW)

    comb_in = nc.dram_tensor("comb", [M, ES], f32, kind="ExternalInput")
    qp_in = nc.dram_tensor("qp", [128, T, 3], f32, kind="ExternalInput")
    idx_in = nc.dram_tensor("idx16", [128, T, GPT, NI // 16], i16,
                            kind="ExternalInput")
    w1b_in = nc.dram_tensor("w1b", [8, 128], bf16, kind="ExternalInput")
    b1b_in = nc.dram_tensor("b1b", [128, 1], f32, kind="ExternalInput")
    w2b_in = nc.dram_tensor("w2b", [128, 128], bf16, kind="ExternalInput")
    b2b_in = nc.dram_tensor("b2b", [128, 1], f32, kind="ExternalInput")
    w3b_in = nc.dram_tensor("w3b", [128, 128], f32, kind="ExternalInput")
    wgb_in = nc.dram_tensor("wgb", [128, 3, 128], f32, kind="ExternalInput")
    bgb_in = nc.dram_tensor("bgb", [128, 3], f32, kind="ExternalInput")
    wvb_in = nc.dram_tensor("wvb", [128, 3, 128], bf16, kind="ExternalInput")
    ident_in = nc.dram_tensor("ident", [128, 128], f32, kind="ExternalInput")
    out_dev = nc.dram_tensor("out", [3, OUT, NQ], f32, kind="ExternalOutput")

    with tile.TileContext(nc) as tc, ExitStack() as ctx:
        const = ctx.enter_context(tc.tile_pool(name="const", bufs=1))
        gpool = ctx.enter_context(tc.tile_pool(name="gpool", bufs=3))
        tpool = ctx.enter_context(tc.tile_pool(name="tpool", bufs=1))
        sfpool = ctx.enter_context(tc.tile_pool(name="sfpool", bufs=2))
        pnpool = ctx.enter_context(tc.tile_pool(name="pnpool", bufs=2))
        planes = ctx.enter_context(tc.tile_pool(name="planes", bufs=2))
        temps = ctx.enter_context(tc.tile_pool(name="temps", bufs=2))
        mlpp = ctx.enter_context(tc.tile_pool(name="mlpp", bufs=1))
        small = ctx.enter_context(tc.tile_pool(name="small", bufs=2))
        psmlp = ctx.enter_context(tc.tile_pool(name="psmlp", bufs=3, space="PSUM"))
        pssm = ctx.enter_context(tc.tile_pool(name="pssm", bufs=2, space="PSUM"))

        def cload(name, dram, shape, dt=f32):
            t = const.tile(shape, dt, tag=name, name=name)
            if len(shape) > 3:
                dims = " ".join(f"d{i}" for i in range(len(shape) - 1))
                pat = f"p {dims} -> p ({dims})"
                nc.sync.dma_start(t[:].rearrange(pat), dram.ap().rearrange(pat))
            else:
                nc.sync.dma_start(t[:], dram.ap())
            return t

        qp_t = cload("qp", qp_in, [128, T, 3])
        idx_t = cload("idx16", idx_in, [128, T, GPT, NI // 16], i16)
        w1b_t = cload("w1b", w1b_in, [8, 128], bf16)
        b1b_t = cload("b1b", b1b_in, [128, 1])
        w2b_t = cload("w2b", w2b_in, [128, 128], bf16)
        b2b_t = cload("b2b", b2b_in, [128, 1])
        w3b_t = cload("w3b", w3b_in, [128, 128])
        wgb_t = cload("wgb", wgb_in, [128, 3, 128])
        bgb_t = cload("bgb", bgb_in, [128, 3])
        wvb_t = cload("wvb", wvb_in, [128, 3, 128], bf16)
        ident_t = cload("ident", ident_in, [128, 128])

        out_re = out_dev.ap().rearrange("c (jj p) q -> p c jj q", jj=3)
        TT = nc.vector.tensor_tensor
        STT = nc.vector.scalar_tensor_tensor

        RW = 128 * K        # MLP rows per query tile (4096)
        HC = RW // 2        # rows per hh half (2048)
        gctr = 0

        for j in range(NPAIR):
            # ---- gather the pair's 2*128*K neighbor rows ----
            gt = gpool.tile([128, 2, K, ES], f32, tag="gt", name="gt")
            for t2 in range(2):
                for g in range(GPT):
                    nc.gpsimd.dma_gather(
                        out_ap=gt[:, t2, g * KPG : (g + 1) * KPG, :],
                        in_ap=comb_in.ap(),
                        idxs_ap=idx_t[:, 2 * j + t2, g, :],
                        num_idxs=NI,
                        num_idxs_reg=NI,
                        elem_size=ES,
                        queue_num=gctr % NQ_SW,
                    )
                    gctr += 1

            # ---- K-sum of bf16 s_feats: tree adds (contiguous reads) ----
            gtb = gt[:].bitcast(bf16)          # [128, 2, K, 256]
            s16 = tpool.tile([128, 2, 16, 192], bf16, tag="s16")
            TT(s16[:], gtb[:, :, 0:16, 0:192], gtb[:, :, 16:32, 0:192], ALU.add)
            s8 = tpool.tile([128, 2, 8, 192], bf16, tag="s8")
            TT(s8[:], s16[:, :, 0:8, :], s16[:, :, 8:16, :], ALU.add)
            s4 = tpool.tile([128, 2, 4, 192], f32, tag="s4")
            TT(s4[:], s8[:, :, 0:4, :], s8[:, :, 4:8, :], ALU.add)
            s2 = tpool.tile([128, 2, 2, 192], f32, tag="s2")
            TT(s2[:], s4[:, :, 0:2, :], s4[:, :, 2:4, :], ALU.add)
            sfs = sfpool.tile([128, 2, 192], f32, tag="sfs")
            TT(sfs[:], s2[:, :, 0, :], s2[:, :, 1, :], ALU.add)

            # ---- pack pts/normals/|n|^2 for the pair (ACT copy) ----
            pnb = pnpool.tile([128, 2, K, 8], f32, tag="pnb")
            nc.scalar.copy(pnb[:, :, :, 0:7], gt[:, :, :, PNO : PNO + 7])

            # ---- PPF geometric features ([128, 2, K] planes) ----
            def ptile(tag):
                return planes.tile([128, 2, K], bf16, tag=tag, name=tag)

            def ttile(tag):
                return temps.tile([128, 2, K], f32, tag=tag, name=tag)

            def np_c(c):
                return pnb[:, :, :, c]

            def nn_c(c):
                return pnb[:, :, :, 3 + c]

            def qn_c(c):
                return pnb[:, :, 0, 3 + c].to_broadcast([128, 2, K])

            def qp_c(c):
                return qp_t[:, 2 * j : 2 * j + 2, c].to_broadcast([128, 2, K])

            vd = []
            for c in range(3):
                t_ = ttile(f"vd{c}")
                TT(t_[:], np_c(c), qp_c(c), ALU.subtract)
                vd.append(t_)

            def dot_views(av, bv, out_tag):
                m0 = ttile("dm0")
                TT(m0[:], av[0], bv[0], ALU.mult)
                m1 = ttile("dm1")
                TT(m1[:], av[1], bv[1], ALU.mult)
                s = ttile(out_tag)
                TT(s[:], m0[:], m1[:], ALU.add)
                m2 = ttile("dm0")
                TT(m2[:], av[2], bv[2], ALU.mult)
                TT(s[:], s[:], m2[:], ALU.add)
                return s

            vdv = [t_[:] for t_ in vd]
            qnv = [qn_c(c) for c in range(3)]
            nnv = [nn_c(c) for c in range(3)]
            qn2 = pnb[:, :, 0, 6].to_broadcast([128, 2, K])
            nn2 = pnb[:, :, :, 6]

            dd = dot_views(vdv, vdv, "dd")
            ys = []
            for i, (av, bv) in enumerate(((qnv, vdv), (nnv, vdv), (qnv, nnv))):
                ys.append(dot_views(av, bv, f"y{i}"))

            # |a x b|^2 = |a|^2 |b|^2 - (a.b)^2  (Lagrange), clamped at 0
            ysq = []
            for i in range(3):
                sq = ttile(f"ysq{i}")
                nc.scalar.square(sq[:], ys[i][:])
                ysq.append(sq)
            rss = []
            for i, nrm2 in enumerate((qn2, nn2, None)):
                rs = ttile(f"rs{i}")
                if i < 2:
                    TT(rs[:], nrm2, dd[:], ALU.mult)
                else:
                    TT(rs[:], qn2, nn2, ALU.mult)
                TT(rs[:], rs[:], ysq[i][:], ALU.subtract)
                nc.vector.tensor_scalar(rs[:], rs[:], 0.0, None, ALU.max)
                rss.append(rs)

            # sqrt-table phase: 4 sqrts + 3 signs
            d_pl = ptile("d_pl")
            nc.scalar.activation(d_pl[:], dd[:], AF.Sqrt)
            rs_r = []
            for i in range(3):
                r = ttile(f"r{i}")
                nc.scalar.activation(r[:], rss[i][:], AF.Sqrt)
                rs_r.append(r)
            sgns = []
            for i in range(3):
                sg = ttile(f"sg{i}")
                nc.scalar.sign(sg[:], ys[i][:])
                sgns.append(sg)

            # arctan-table phase
            a_pls = [d_pl]
            for i in range(3):
                iy = ttile(f"iy{i}")
                nc.vector.reciprocal(iy[:], ys[i][:])
                tq = ttile("dm0")
                TT(tq[:], rs_r[i][:], iy[:], ALU.mult)
                at = ttile("dm1")
                nc.scalar.activation(at[:], tq[:], AF.Arctan)
                pl = ptile(f"a{i}_pl")
                STT(pl[:], sgns[i][:], -PI / 2, at[:], ALU.mult, ALU.add)
                a_pls.append(pl)

            # ---- pack planes into MLP rows: pf[8, 4096] bf16 ----
            pf = mlpp.tile([8, RW], bf16, tag="pf", bufs=2)
            for t2 in range(2):
                for ci in range(4):
                    nc.sync.dma_start(
                        pf[t2 * 4 + ci : t2 * 4 + ci + 1, :],
                        a_pls[ci][:, t2, :],
                    )

            # ---- MLP (block-diagonal 2-tile packing) ----
            ksum = small.tile([128, 128], f32, tag="ksum")
            for hh in range(2):
                h1s = mlpp.tile([128, HC], bf16, tag="h1s", bufs=2)
                for ch in range(HC // 512):
                    sl = slice(ch * 512, (ch + 1) * 512)
                    slg = slice(hh * HC + ch * 512, hh * HC + (ch + 1) * 512)
                    h1p = psmlp.tile([128, 512], f32, tag="psmlp")
                    nc.tensor.matmul(
                        h1p[:], w1b_t[:], pf[:, slg], start=True, stop=True
                    )
                    nc.scalar.activation(
                        h1s[:, sl], h1p[:], AF.Relu, bias=b1b_t[:]
                    )
                h2s = mlpp.tile([128, HC], bf16, tag="h2s", bufs=2)
                for ch in range(HC // 512):
                    sl = slice(ch * 512, (ch + 1) * 512)
                    h2p = psmlp.tile([128, 512], f32, tag="psmlp")
                    nc.tensor.matmul(
                        h2p[:], w2b_t[:], h1s[:, sl], start=True, stop=True
                    )
                    nc.scalar.activation(
                        h2s[:, sl], h2p[:], AF.Relu, bias=b2b_t[:]
                    )
                nc.vector.reduce_sum(
                    ksum[:, hh * 64 : (hh + 1) * 64],
                    h2s[:].rearrange("p (q k) -> p q k", k=K),
                    mybir.AxisListType.X,
                )

            pmp = pssm.tile([128, 128], f32, tag="pssm")
            nc.tensor.matmul(pmp[:], w3b_t[:], ksum[:], start=True, stop=True)
            pms = small.tile([128, 128], f32, tag="pms")
            nc.scalar.copy(pms[:], pmp[:])  # b3 folded into bgb on host

            gates = []
            for jj in range(3):
                gp = pssm.tile([128, 128], f32, tag="pssm")
                nc.tensor.matmul(
                    gp[:], wgb_t[:, jj, :], pms[:], start=True, stop=True
                )
                gs = small.tile([128, 128], f32, tag=f"gate{jj}", name=f"gate{jj}")
                nc.scalar.activation(
                    gs[:], gp[:], AF.Sigmoid, bias=bgb_t[:, jj : jj + 1]
                )
                gates.append(gs)

            # ---- value path: transpose sfsum, then batched Wv matmuls ----
            av = sfs[:].rearrange("p t (d c) -> p c (t d)", c=3)
            aggs = small.tile([128, 3, 128], bf16, tag="aggs")
            for c in range(3):
                tp = pssm.tile([128, 128], f32, tag="pssm")
                nc.tensor.transpose(tp[:], av[:, c, :], ident_t[:])
                nc.scalar.copy(aggs[:, c, :], tp[:])
            vstage = small.tile([128, 3, 3, 128], f32, tag="vstage")
            for jj in range(3):
                vp = pssm.tile([128, 3, 128], f32, tag="psv")
                nc.tensor.matmul(
                    vp[:].rearrange("p c q -> p (c q)"),
                    wvb_t[:, jj, :],
                    aggs[:].rearrange("p c q -> p (c q)"),
                    start=True, stop=True,
                )
                for c in range(3):
                    TT(vstage[:, c, jj, :], vp[:, c, :], gates[jj][:], ALU.mult)

            for h in range(2):
                q0 = (2 * j + h) * 128
                nc.sync.dma_start(
                    out_re[:, :, :, q0 : q0 + 128].rearrange(
                        "p c jj q -> p (c jj) q"
                    ),
                    vstage[h * 64 : (h + 1) * 64, :, :, :].rearrange(
                        "p c jj q -> p (c jj) q"
                    ),
                )

    nc.compile()
    return nc


def _f32_to_bf16_bits(x):
    """Round-to-nearest-even f32 -> bf16, returned as uint16 bits."""
    u = np.ascontiguousarray(x, dtype=np.float32).view(np.uint32)
    rounded = (u + 0x7FFF + ((u >> 16) & 1)) >> 16
    return rounded.astype(np.uint16)


def _host_prep(q_pts, s_pts, s_feats, neighbor_indices, normals,
               W1, b1, W2, b2, W3, b3, Wg, bg, Wv, T, n_total=N):
    NQ = 128 * T
    n_per_core = n_total // N_CORES
    f = np.float32
    bf = ml_dtypes.bfloat16

    comb = np.zeros((M, ES), dtype=f)
    cb = comb.view(np.uint16).reshape(M, ES * 2)
    cb[:, : 2 * SFW] = _f32_to_bf16_bits(s_feats.reshape(M, 192))
    comb[:, PNO : PNO + 3] = s_pts
    comb[:, PNO + 3 : PNO + 6] = normals
    comb[:, PNO + 6] = (normals.astype(f) ** 2).sum(axis=-1)

    W1T = W1.T.astype(f).copy()
    W1T[1:4] *= f(1.0 / PI)
    w1b = np.zeros((8, 128), dtype=f)
    w1b[0:4, 0:64] = W1T
    w1b[4:8, 64:128] = W1T
    # atan2 via sign: constant pi/2 * (sum of folded angle columns) -> b1
    b1_eff = b1.astype(f) + f(PI / 2) * W1T[1:4].sum(axis=0)
    b1b = np.concatenate([b1_eff, b1_eff]).astype(f)[:, None]

    def blockdiag2(A):
        n_, m_ = A.shape
        o = np.zeros((2 * n_, 2 * m_), dtype=f)
        o[:n_, :m_] = A
        o[n_:, m_:] = A
        return o

    w2b = blockdiag2(W2.T.astype(f))
    b2b = np.concatenate([b2, b2]).astype(f)[:, None]
    w3b = blockdiag2((W3.T / K).astype(f))
    gb3 = Wg.astype(f) @ b3.astype(f)  # b3 folded through the gate projection

    WgT = Wg.T.astype(f)
    WvT = (Wv.T / K).astype(f)
    wgb = np.zeros((3, 128, 128), dtype=f)
    wvb = np.zeros((3, 128, 128), dtype=f)
    bgb = np.zeros((128, 3), dtype=f)
    for jj in range(3):
        wgb[jj] = blockdiag2(WgT[:, jj * 64 : (jj + 1) * 64])
        wvb[jj] = blockdiag2(WvT[:, jj * 64 : (jj + 1) * 64])
        bgb[:, jj] = np.concatenate(
            [(bg + gb3)[jj * 64 : (jj + 1) * 64]] * 2
        )
    wgb_host = np.ascontiguousarray(wgb.transpose(1, 0, 2))
    wvb_host = np.ascontiguousarray(wvb.transpose(1, 0, 2)).astype(bf)
    ident = np.eye(128, dtype=f)

    shared = dict(
        comb=comb, w1b=w1b.astype(bf), b1b=b1b, w2b=w2b.astype(bf), b2b=b2b,
        w3b=w3b, wgb=wgb_host, bgb=bgb, wvb=wvb_host, ident=ident,
    )

    in_maps = []
    for i in range(N_CORES):
        lo = i * n_per_core
        hi = lo + n_per_core
        qp_pad = np.zeros((NQ, 3), dtype=f)
        qp_pad[: hi - lo] = q_pts[lo:hi]
        idx_pad = np.zeros((NQ, K), dtype=np.int64)
        idx_pad[: hi - lo] = neighbor_indices[lo:hi]

        qp_host = np.ascontiguousarray(qp_pad.reshape(T, 128, 3).transpose(1, 0, 2))

        # idx16[p, t, g, s]: gather g of tile t covers logical rows
        # i' = (k - g*KPG)*128 + q, wrapped: w[l, s] = list[s*16 + l]
        idx16 = np.zeros((128, T, GPT, NI // 16), np.int16)
        for t in range(T):
            arr = idx_pad[t * 128 : (t + 1) * 128, :]      # [128 q, K]
            for g in range(GPT):
                lst = arr[:, g * KPG : (g + 1) * KPG].T.reshape(NI)
                idx16[:, t, g, :] = np.tile(
                    lst.reshape(NI // 16, 16).T.astype(np.int16), (8, 1)
                )

        m = dict(shared)
        m.update(qp=qp_host, idx16=idx16)
        in_maps.append(m)
    return in_maps


def kernel(**inputs):
    from concourse.bass_utils import run_bass_kernel_spmd

    T = 20
    inputs = {k: np.asarray(v) for k, v in inputs.items()}
    idx = inputs["neighbor_indices"].astype(np.int64)

    if T not in _NC_CACHE:
        _NC_CACHE[T] = _build_nc(T)
    nc = _NC_CACHE[T]

    in_maps = _host_prep(
        inputs["q_pts"], inputs["s_pts"], inputs["s_feats"], idx,
        inputs["normals"], inputs["W1"], inputs["b1"], inputs["W2"],
        inputs["b2"], inputs["W3"], inputs["b3"], inputs["Wg"],
        inputs["bg"], inputs["Wv"], T,
    )
    res = run_bass_kernel_spmd(nc, in_maps, core_ids=list(range(N_CORES)))

    n_per_core = N // N_CORES
    out = np.empty((N, OUT, 3), dtype=np.float32)
    for i in range(N_CORES):
        o = res.results[i]["out"]
        out[i * n_per_core : (i + 1) * n_per_core] = o.transpose(2, 1, 0)[:n_per_core]
    return out
